# revision 2
# baseline (speedup 1.0000x reference)
"""Bass/Trainium2 kernel for GruAttCosMeanNet (nn_GruAttCosMeanNet_39591008535146).

Data-parallel over batch: 8 cores x 2 batch rows each.
Per core: bidirectional GRU encoders (context len 128, 5 options len 64),
Bahdanau additive attention per option, attention GRUs over the aggregated
sequences, cosine similarity.  Final softmax over 5 options is done on host
(16x5, negligible).

Device layouts (per core, p = SBUF partition):
  - GRU state/gates: [3H on partitions as 6 tiles of 128, batch cols on free]
  - recurrence matmul: stationary = Wh^T k-tile (bf16, FWL), moving = h cols
  - encoder outputs stored transposed [h-dim part, (t, col)] in bf16
  - attention energies: s[h, (q,c)] = tanh(optq + ctxk) built with
    broadcast APs on VE, tanh on SE, then e[c,q] via PE with s as stationary
    and v as the 1-column moving operand.
"""
import sys
sys.path.insert(0, "/opt/trn_rl_repo")
import numpy as np
import ml_dtypes

import concourse.bass as bass
import concourse.mybir as mybir
import concourse.tile as tile
from concourse import bacc, bass_utils
from concourse.masks import make_identity

BF16 = mybir.dt.bfloat16
F32 = mybir.dt.float32
AF = mybir.ActivationFunctionType
ALU = mybir.AluOpType

B, LC, LO, NOPT, E, H = 16, 128, 64, 5, 300, 256
NCORES = 8
BL = B // NCORES          # 2 batch rows per core
NI = BL * NOPT            # 10 (b,opt) pairs per core
NBM = BL + NI             # 12 cols in main GRU (2 ctx + 10 opt)
NBA = 2 * NI              # 20 cols in att GRU (10 actx + 10 aopt)
H3 = 3 * H                # 768
bf = ml_dtypes.bfloat16

_CACHE = {}


def _build():
    nc = bacc.Bacc("TRN2", target_bir_lowering=False, debug=False,
                   num_devices=NCORES)

    d = {}
    d["xtc"] = nc.dram_tensor("xtc", [2, 3, 128, LC * BL], BF16, kind="ExternalInput")
    d["xto"] = nc.dram_tensor("xto", [2, 3, 128, LO * NI], BF16, kind="ExternalInput")
    d["wir"] = nc.dram_tensor("wir", [2, 3, 128, H3], BF16, kind="ExternalInput")
    d["whr"] = nc.dram_tensor("whr", [2, 2, 128, H3], BF16, kind="ExternalInput")
    d["wia"] = nc.dram_tensor("wia", [2, 3, 128, H3], BF16, kind="ExternalInput")
    d["wha"] = nc.dram_tensor("wha", [2, 2, 128, H3], BF16, kind="ExternalInput")
    d["wk"] = nc.dram_tensor("wk", [4, 128, H], BF16, kind="ExternalInput")
    d["wq"] = nc.dram_tensor("wq", [4, 128, H], BF16, kind="ExternalInput")
    d["bhn_r"] = nc.dram_tensor("bhn_r", [128, 2, 2], F32, kind="ExternalInput")
    d["bhn_a"] = nc.dram_tensor("bhn_a", [128, 2, 2], F32, kind="ExternalInput")
    d["v"] = nc.dram_tensor("v", [128, 2], F32, kind="ExternalInput")
    d["out"] = nc.dram_tensor("out", [1, NI], F32, kind="ExternalOutput")

    with tile.TileContext(nc) as tc:
        _body(nc, tc, d)
    nc.compile()
    return nc


def _body(nc, tc, d):
    import contextlib
    ctx = contextlib.ExitStack()
    with ctx:
        consts = ctx.enter_context(tc.tile_pool(name="consts", bufs=1))
        wpool = ctx.enter_context(tc.tile_pool(name="weights", bufs=1))
        xppool = ctx.enter_context(tc.tile_pool(name="xp", bufs=1))
        encp = ctx.enter_context(tc.tile_pool(name="enc", bufs=1))
        hpool = ctx.enter_context(tc.tile_pool(name="hstate", bufs=1))
        spool = ctx.enter_context(tc.tile_pool(name="spool", bufs=2))
        small = ctx.enter_context(tc.tile_pool(name="small", bufs=3))
        psg = ctx.enter_context(tc.tile_pool(name="psg", bufs=3, space="PSUM"))
        psum_hp = ctx.enter_context(tc.tile_pool(name="pshp", bufs=2, space="PSUM"))
        psum_e = ctx.enter_context(tc.tile_pool(name="pse", bufs=2, space="PSUM"))

        def ps_tile(shape):
            return psg.tile(shape, F32, tag="ps", name="pst")

        # ---- constants / weights ----
        ident = consts.tile([128, 128], F32)
        make_identity(nc, ident[:])
        ones128 = consts.tile([128, 1], F32)
        nc.vector.memset(ones128[:], 1.0)

        wir = wpool.tile([128, 2, 3, H3], BF16)
        whr = wpool.tile([128, 2, 2, H3], BF16)
        wia = wpool.tile([128, 2, 3, H3], BF16)
        wha = wpool.tile([128, 2, 2, H3], BF16)
        wk = wpool.tile([128, 4, H], BF16)
        wq = wpool.tile([128, 4, H], BF16)
        bhn_r = consts.tile([128, 2, 2], F32)
        bhn_a = consts.tile([128, 2, 2], F32)
        vsb = consts.tile([128, 2], F32)
        for dd in range(2):
            for k in range(3):
                nc.sync.dma_start(wir[:, dd, k, :], d["wir"].ap()[dd, k])
                nc.sync.dma_start(wia[:, dd, k, :], d["wia"].ap()[dd, k])
            for k in range(2):
                nc.sync.dma_start(whr[:, dd, k, :], d["whr"].ap()[dd, k])
                nc.sync.dma_start(wha[:, dd, k, :], d["wha"].ap()[dd, k])
        for k in range(4):
            nc.sync.dma_start(wk[:, k, :], d["wk"].ap()[k])
            nc.sync.dma_start(wq[:, k, :], d["wq"].ap()[k])
        nc.sync.dma_start(bhn_r[:], d["bhn_r"].ap())
        nc.sync.dma_start(bhn_a[:], d["bhn_a"].ap())
        nc.sync.dma_start(vsb[:], d["v"].ap())

        xtc = wpool.tile([128, 2, 3, LC * BL], BF16)
        xto = wpool.tile([128, 2, 3, LO * NI], BF16)
        for dd in range(2):
            for k in range(3):
                nc.sync.dma_start(xtc[:, dd, k, :], d["xtc"].ap()[dd, k])
                nc.sync.dma_start(xto[:, dd, k, :], d["xto"].ap()[dd, k])

        # ======== Phase 1: main GRU input projections ========
        # unified xp: [p, dir, gate, t, col]; opt cols zero-padded outside
        # their valid range; bwd opt block stored at t in [64,128) so the
        # uniform bwd index T-1-t_f reads opt time 63-t_f.
        xpm = xppool.tile([128, 2, 6, LC, NBM], BF16, tag="xpu")
        nc.vector.memset(xpm[:, 0, :, LO:, BL:], 0.0)
        nc.vector.memset(xpm[:, 1, :, :LO, BL:], 0.0)

        def proj_main(groups):
            for (xsrc, dd, tb, cl, ch, T2, nbg, tch) in groups:
                for jg in range(6):
                    for t0 in range(0, T2, tch):
                        tw = min(tch, T2 - t0)
                        cw = tw * nbg
                        pt = ps_tile([128, 512])
                        for k in range(3):
                            nc.tensor.matmul(
                                pt[:, :cw],
                                wir[:, dd, k, jg * 128:(jg + 1) * 128],
                                xsrc[:, k, t0 * nbg:t0 * nbg + cw],
                                start=(k == 0), stop=(k == 2))
                        nc.scalar.copy(
                            xpm[:, dd, jg, tb + t0:tb + t0 + tw, cl:ch],
                            pt[:, :cw])

        proj_main([
            (xtc[:, 0], 0, 0, 0, BL, LC, BL, 128),
            (xtc[:, 1], 1, 0, 0, BL, LC, BL, 128),
            (xto[:, 0], 0, 0, BL, NBM, LO, NI, 32),
            (xto[:, 1], 1, LO, BL, NBM, LO, NI, 32),
        ])

        # ======== Phase 2/6 shared: one bidirectional GRU time step ========
        def gru_step(t_f, whx, xps, hst, nb, bhn, store):
            """xps: list of (xp_tile, npt, c0, Tb); cols [c0, c0+npt) of the
            batch take input from xp_tile at their own time index."""
            for dd in range(2):
                hp = psum_hp.tile([128, 6, nb], F32, tag="hp")
                for jg in range(6):
                    for k in range(2):
                        nc.tensor.matmul(
                            hp[:, jg, :],
                            whx[:, dd, k, jg * 128:(jg + 1) * 128],
                            hst[:, dd, k, :],
                            start=(k == 0), stop=(k == 1))
                rz = small.tile([128, 4, nb], F32, tag="rz")
                if len(xps) == 1:
                    xp, npt, c0, Tb = xps[0]
                    t2 = t_f if dd == 0 else Tb - 1 - t_f
                    nc.vector.tensor_tensor(
                        rz[:], hp[:, 0:4, :], xp[:, dd, 0:4, t2, :], ALU.add)
                else:
                    nc.vector.tensor_copy(rz[:], hp[:, 0:4, :])
                    for (xp, npt, c0, Tb) in xps:
                        t2 = t_f if dd == 0 else Tb - 1 - t_f
                        if not (0 <= t2 < Tb):
                            continue
                        nc.vector.tensor_tensor(
                            rz[:, :, c0:c0 + npt], rz[:, :, c0:c0 + npt],
                            xp[:, dd, 0:4, t2, :], ALU.add)
                nc.scalar.activation(rz[:], rz[:], AF.Sigmoid)
                nt = small.tile([128, 2, nb], F32, tag="nt")
                nc.vector.tensor_scalar(
                    nt[:, 0, :], hp[:, 4, :], bhn[:, dd, 0:1], None, op0=ALU.add)
                nc.vector.tensor_scalar(
                    nt[:, 1, :], hp[:, 5, :], bhn[:, dd, 1:2], None, op0=ALU.add)
                nc.vector.tensor_tensor(nt[:], rz[:, 0:2, :], nt[:], ALU.mult)
                for (xp, npt, c0, Tb) in xps:
                    t2 = t_f if dd == 0 else Tb - 1 - t_f
                    if len(xps) > 1 and not (0 <= t2 < Tb):
                        continue
                    nc.vector.tensor_tensor(
                        nt[:, :, c0:c0 + npt], nt[:, :, c0:c0 + npt],
                        xp[:, dd, 4:6, t2, :], ALU.add)
                nc.scalar.activation(nt[:], nt[:], AF.Tanh)
                hn = small.tile([128, 2, nb], F32, tag="hn")
                nc.vector.tensor_tensor(hn[:], hst[:, dd, :, :], nt[:],
                                        ALU.subtract)
                nc.vector.tensor_tensor(hn[:], rz[:, 2:4, :], hn[:], ALU.mult)
                nc.vector.tensor_tensor(hst[:, dd, :, :], nt[:], hn[:], ALU.add)
                store(dd, t_f, hst)

        # ======== Phase 2: main GRU recurrence ========
        ence = encp.tile([128, 4, LC, BL], BF16)
        enco = encp.tile([128, 4, LO, NI], BF16)
        hm = hpool.tile([128, 2, 2, NBM], BF16, tag="h")
        nc.vector.memset(hm[:], 0.0)

        def store_main(dd, t_f, hst):
            tc_ = t_f if dd == 0 else LC - 1 - t_f
            nc.vector.tensor_copy(ence[:, 2 * dd:2 * dd + 2, tc_, :],
                                  hst[:, dd, :, 0:BL])
            to = t_f if dd == 0 else LO - 1 - t_f
            if 0 <= to < LO:
                nc.vector.tensor_copy(enco[:, 2 * dd:2 * dd + 2, to, :],
                                      hst[:, dd, :, BL:])

        xps_main = [(xpm, NBM, 0, LC)]
        for t in range(LC):
            gru_step(t, whr, xps_main, hm, NBM, bhn_r, store_main)

        # ======== Phase 3: ctx_key / opt_q projections ========
        ctxkT = encp.tile([128, 2, LC, BL], F32)
        optqT = encp.tile([128, 2, LO, NI], F32)

        def kq(dst, w, src, T, nb2, tch):
            for jg in range(2):
                for t0 in range(0, T, tch):
                    cw = (min(tch, T - t0)) * nb2
                    pt = ps_tile([128, 512])
                    for k in range(4):
                        nc.tensor.matmul(
                            pt[:, :cw], w[:, k, jg * 128:(jg + 1) * 128],
                            src[:, k, t0:t0 + min(tch, T - t0), :],
                            start=(k == 0), stop=(k == 3))
                    nc.vector.tensor_copy(
                        dst[:, jg, t0:t0 + min(tch, T - t0), :], pt[:, :cw])

        kq(ctxkT, wk, ence, LC, BL, 128)       # 128*2=256 cols/chunk
        kq(optqT, wq, enco, LO, NI, 32)        # 32*10=320 cols/chunk

        ctxk_cb = [[None, None] for _ in range(BL)]
        for b in range(BL):
            for jg in range(2):
                pt = ps_tile([128, 128])
                nc.tensor.transpose(pt[:, :128], ctxkT[:, jg, :, b], ident[:])
                sb = small.tile([128, 128], BF16, tag=f"ck{b}{jg}")
                nc.vector.tensor_copy(sb[:], pt[:, :128])
                ctxk_cb[b][jg] = sb

        # ======== Phase 4: attention per (b, opt) ========
        actxT = encp.tile([128, 2, NI, LC], BF16)
        aoptT = encp.tile([128, 2, NI, LO], BF16)
        QCH = 16
        for b in range(BL):
            for o in range(NOPT):
                i = b * NOPT + o
                e_ps = psum_e.tile([128, LO], F32, tag="e")
                for q0 in range(0, LO, QCH):
                    sts = []
                    for jg in range(2):
                        st = spool.tile([128, QCH, LC], F32, tag=f"s{jg}")
                        nc.vector.tensor_tensor(
                            st[:],
                            optqT[:, jg, q0:q0 + QCH, i:i + 1]
                                .broadcast_to([128, QCH, LC]),
                            ctxkT[:, jg, None, :, b]
                                .broadcast_to([128, QCH, LC]),
                            ALU.add)
                        nc.scalar.activation(st[:], st[:], AF.Tanh)
                        sts.append(st)
                    for q in range(QCH):
                        for jg in range(2):
                            nc.tensor.matmul(
                                e_ps[:, q0 + q:q0 + q + 1],
                                sts[jg][:, q, :], vsb[:, jg:jg + 1],
                                start=(jg == 0), stop=(jg == 1))
                # softmax over q (free axis of e[c,q]) -> P1
                e_cq = small.tile([128, LO], F32, tag="ecq")
                nc.vector.tensor_copy(e_cq[:], e_ps[:])
                mx = small.tile([128, 1], F32, tag="mx")
                nc.vector.tensor_reduce(mx[:], e_cq[:],
                                        axis=mybir.AxisListType.X, op=ALU.max)
                nc.vector.tensor_scalar_mul(mx[:], mx[:], -1.0)
                p1 = small.tile([128, LO], F32, tag="p1")
                nc.scalar.activation(p1[:], e_cq[:], AF.Exp, bias=mx[:])
                sm = small.tile([128, 1], F32, tag="sm")
                nc.vector.tensor_reduce(sm[:], p1[:],
                                        axis=mybir.AxisListType.X, op=ALU.add)
                nc.vector.reciprocal(sm[:], sm[:])
                nc.vector.tensor_scalar_mul(p1[:], p1[:], sm[:])
                pt1 = ps_tile([128, 512])
                nc.tensor.transpose(pt1[:64, :128], p1[:], ident[:])
                p1t = small.tile([64, 128], BF16, tag="p1tb")
                nc.vector.tensor_copy(p1t[:], pt1[:64, :128])
                # e^T -> softmax over c -> P2
                pt2 = ps_tile([128, 512])
                nc.tensor.transpose(pt2[:64, :128], e_cq[:], ident[:])
                e_qc = small.tile([64, 128], F32, tag="eqc")
                nc.vector.tensor_copy(e_qc[:], pt2[:64, :128])
                mx2 = small.tile([64, 1], F32, tag="mx2")
                nc.vector.tensor_reduce(mx2[:], e_qc[:],
                                        axis=mybir.AxisListType.X, op=ALU.max)
                nc.vector.tensor_scalar_mul(mx2[:], mx2[:], -1.0)
                p2 = small.tile([64, 128], F32, tag="p2")
                nc.scalar.activation(p2[:], e_qc[:], AF.Exp, bias=mx2[:])
                sm2 = small.tile([64, 1], F32, tag="sm2")
                nc.vector.tensor_reduce(sm2[:], p2[:],
                                        axis=mybir.AxisListType.X, op=ALU.add)
                nc.vector.reciprocal(sm2[:], sm2[:])
                nc.vector.tensor_scalar_mul(p2[:], p2[:], sm2[:])
                pt3 = ps_tile([128, 512])
                nc.tensor.transpose(pt3[:, :64], p2[:], ident[:64, :64])
                p2t = small.tile([128, 64], BF16, tag="p2tb")
                nc.vector.tensor_copy(p2t[:], pt3[:, :64])
                for jg in range(2):
                    pt4 = ps_tile([128, 512])
                    nc.tensor.transpose(pt4[:64, :128], optqT[:, jg, :, i],
                                        ident[:])
                    oq = small.tile([64, 128], BF16, tag=f"oqb{jg}")
                    nc.vector.tensor_copy(oq[:], pt4[:64, :128])
                    ac_ps = ps_tile([128, 512])
                    nc.tensor.matmul(ac_ps[:, :128], oq[:], p1t[:],
                                     start=True, stop=True)
                    nc.vector.tensor_copy(actxT[:, jg, i, :], ac_ps[:, :128])
                    ao_ps = ps_tile([128, 512])
                    nc.tensor.matmul(ao_ps[:, :64], ctxk_cb[b][jg][:], p2t[:],
                                     start=True, stop=True)
                    nc.vector.tensor_copy(aoptT[:, jg, i, :], ao_ps[:, :64])

        # ======== Phase 5: att GRU input projections ========
        xpac = xppool.tile([128, 2, 6, LC, NI], BF16, tag="xpc")
        xpao = xppool.tile([128, 2, 6, LO, NI], BF16, tag="xpd")
        onesrow = consts.tile([1, LC * NI], BF16)
        nc.vector.memset(onesrow[:], 1.0)

        def proj_att(dst, src, T, tch):
            for dd in range(2):
                for jg in range(6):
                    for t0 in range(0, T, tch):
                        tw = min(tch, T - t0)
                        cw = tw * NI
                        pt = ps_tile([128, 512])
                        for k in range(2):
                            nc.tensor.matmul(
                                pt[:, :cw],
                                wia[:, dd, k, jg * 128:(jg + 1) * 128],
                                src[:, k, t0:t0 + tw, :],
                                start=(k == 0), stop=False)
                        nc.tensor.matmul(
                            pt[:, :cw],
                            wia[0:1, dd, 2, jg * 128:(jg + 1) * 128],
                            onesrow[0:1, :cw],
                            start=False, stop=True)
                        nc.scalar.copy(dst[:, dd, jg, t0:t0 + tw, :],
                                       pt[:, :cw])

        # transposed views [128, k, t, i] of actxT/aoptT ([128, jg, i, t])
        acv = actxT[:].transpose([0, 1, 3, 2])
        aov = aoptT[:].transpose([0, 1, 3, 2])
        proj_att(xpac, acv, LC, 32)
        proj_att(xpao, aov, LO, 32)

        # ======== Phase 6: att GRU recurrence with mean accumulation ========
        ha = hpool.tile([128, 2, 2, NBA], BF16, tag="h")
        nc.vector.memset(ha[:], 0.0)
        acc_c = encp.tile([128, 2, 2, NI], F32)
        acc_o = encp.tile([128, 2, 2, NI], F32)
        nc.vector.memset(acc_c[:], 0.0)
        nc.vector.memset(acc_o[:], 0.0)

        def store_att(dd, t_f, hst):
            nc.vector.tensor_tensor(acc_c[:, dd], acc_c[:, dd],
                                    hst[:, dd, :, 0:NI], ALU.add)
            to = t_f if dd == 0 else LO - 1 - t_f
            if 0 <= to < LO:
                nc.vector.tensor_tensor(acc_o[:, dd], acc_o[:, dd],
                                        hst[:, dd, :, NI:], ALU.add)

        xps_att = [(xpac, NI, 0, LC), (xpao, NI, NI, LO)]
        for t in range(LC):
            gru_step(t, wha, xps_att, ha, NBA, bhn_a, store_att)

        # ======== Phase 7: cosine similarity ========
        nc.vector.tensor_scalar_mul(acc_c[:], acc_c[:], 1.0 / LC)
        nc.vector.tensor_scalar_mul(acc_o[:], acc_o[:], 1.0 / LO)
        prod = small.tile([128, 2, 2, NI], F32, tag="prod")
        dots_ps = psg.tile([1, 3, 4, NI], F32, tag="ps")
        nc.vector.tensor_tensor(prod[:], acc_c[:], acc_o[:], ALU.mult)
        nc.tensor.matmul(dots_ps[:, 0], ones128[:], prod[:],
                         start=True, stop=True)
        nc.vector.tensor_tensor(prod[:], acc_c[:], acc_c[:], ALU.mult)
        nc.tensor.matmul(dots_ps[:, 1], ones128[:], prod[:],
                         start=True, stop=True)
        nc.vector.tensor_tensor(prod[:], acc_o[:], acc_o[:], ALU.mult)
        nc.tensor.matmul(dots_ps[:, 2], ones128[:], prod[:],
                         start=True, stop=True)
        red = small.tile([1, 3, NI], F32, tag="red")
        nc.vector.tensor_reduce(red[:], dots_ps[:].transpose([0, 1, 3, 2]),
                                axis=mybir.AxisListType.X, op=ALU.add)
        nrm = small.tile([1, NI], F32, tag="nrm")
        nc.vector.tensor_tensor(nrm[:], red[:, 1, :], red[:, 2, :], ALU.mult)
        nc.vector.tensor_scalar_max(nrm[:], nrm[:], 1e-30)
        nc.scalar.activation(nrm[:], nrm[:], AF.Sqrt)
        nc.vector.reciprocal(nrm[:], nrm[:])
        cos = small.tile([1, NI], F32, tag="cos")
        nc.vector.tensor_tensor(cos[:], red[:, 0, :], nrm[:], ALU.mult)
        nc.sync.dma_start(d["out"].ap(), cos[:])


def _prep_inputs(inputs):
    ctx = np.asarray(inputs["context"], np.float32)
    opts = np.asarray(inputs["options"], np.float32)

    def gru_w(pre):
        out = {}
        for dd, sfx in enumerate(("f", "b")):
            out[dd] = {k: np.asarray(inputs[f"{pre}_{k}_{sfx}"], np.float32)
                       for k in ("Wi", "Wh", "bi", "bh")}
        return out

    rnn, att = gru_w("rnn"), gru_w("att")
    Wk = np.asarray(inputs["Wk"], np.float32)
    Wq = np.asarray(inputs["Wq"], np.float32)
    v = np.asarray(inputs["v_energy"], np.float32)

    def wi_pack(g, ein):
        out = np.zeros((2, 3, 128, H3), np.float32)
        for dd in range(2):
            bias = g[dd]["bi"].copy()
            bias[:2 * H] += g[dd]["bh"][:2 * H]
            m = np.zeros((3 * 128, H3), np.float32)
            m[:ein] = g[dd]["Wi"].T
            m[ein] = bias
            out[dd] = m.reshape(3, 128, H3)
        return out.astype(bf)

    def wh_pack(g):
        out = np.zeros((2, 2, 128, H3), np.float32)
        for dd in range(2):
            out[dd] = g[dd]["Wh"].T.reshape(2, 128, H3)
        return out.astype(bf)

    def bhn_pack(g):
        out = np.zeros((128, 2, 2), np.float32)
        for dd in range(2):
            out[:, dd, 0] = g[dd]["bh"][2 * H:2 * H + 128]
            out[:, dd, 1] = g[dd]["bh"][2 * H + 128:]
        return out

    shared = {
        "wir": wi_pack(rnn, E), "whr": wh_pack(rnn),
        "wia": wi_pack(att, H), "wha": wh_pack(att),
        "wk": np.ascontiguousarray(Wk.T.reshape(4, 128, H).astype(bf)),
        "wq": np.ascontiguousarray(Wq.T.reshape(4, 128, H).astype(bf)),
        "bhn_r": np.ascontiguousarray(bhn_pack(rnn)),
        "bhn_a": np.ascontiguousarray(bhn_pack(att)),
        "v": np.ascontiguousarray(v.reshape(2, 128).T.astype(np.float32)),
    }

    in_maps = []
    for c in range(NCORES):
        bs = slice(c * BL, (c + 1) * BL)
        xa = np.zeros((BL, LC, 3 * 128), np.float32)
        xa[:, :, :E] = ctx[bs]
        xa[:, :, E] = 1.0
        xb = np.zeros((NI, LO, 3 * 128), np.float32)
        xb[:, :, :E] = opts[bs].reshape(NI, LO, E)
        xb[:, :, E] = 1.0
        xtc = np.stack([
            xa.transpose(2, 1, 0).reshape(3, 128, LC * BL),
            xa[:, ::-1].transpose(2, 1, 0).reshape(3, 128, LC * BL)]).astype(bf)
        xto = np.stack([
            xb.transpose(2, 1, 0).reshape(3, 128, LO * NI),
            xb[:, ::-1].transpose(2, 1, 0).reshape(3, 128, LO * NI)]).astype(bf)
        m = dict(shared)
        m["xtc"] = np.ascontiguousarray(xtc)
        m["xto"] = np.ascontiguousarray(xto)
        in_maps.append(m)
    return in_maps


def kernel(**inputs):
    if "nc" not in _CACHE:
        _CACHE["nc"] = _build()
    nc = _CACHE["nc"]
    in_maps = _prep_inputs(inputs)
    res = bass_utils.run_bass_kernel_spmd(nc, in_maps,
                                          core_ids=list(range(NCORES)))
    _CACHE["last_exec_ns"] = res.exec_time_ns
    _CACHE["last_res"] = res
    logits = np.concatenate(
        [np.asarray(res.results[c]["out"], np.float32).reshape(BL, NOPT)
         for c in range(NCORES)], axis=0)
    x = logits - logits.max(axis=1, keepdims=True)
    ex = np.exp(x)
    return (ex / ex.sum(axis=1, keepdims=True)).astype(np.float32)


if __name__ == "__main__":
    _build()
    print("build+compile OK")



# revision 18
# speedup vs baseline: 1.3154x; 1.3154x over previous
"""Bass/Trainium2 kernel for GruAttCosMeanNet (nn_GruAttCosMeanNet_39591008535146).

Data-parallel over batch: 8 cores x 2 batch rows each.
Per core: bidirectional GRU encoders (context len 128, 5 options len 64),
Bahdanau additive attention per option, attention GRUs over the aggregated
sequences, cosine similarity.  Final softmax over 5 options is done on host
(16x5, negligible).

Device layouts (per core, p = SBUF partition):
  - GRU state/gates: [3H on partitions as 6 tiles of 128, batch cols on free]
  - recurrence matmul: stationary = Wh^T k-tile (bf16, FWL), moving = h cols
  - encoder outputs stored transposed [h-dim part, (t, col)] in bf16
  - attention energies: s[h, (q,c)] = tanh(optq + ctxk) built with
    broadcast APs on VE, tanh on SE, then e[c,q] via PE with s as stationary
    and v as the 1-column moving operand.
"""
import sys
sys.path.insert(0, "/opt/trn_rl_repo")
import numpy as np
import ml_dtypes

import concourse.bass as bass
import concourse.mybir as mybir
import concourse.tile as tile
from concourse import bacc, bass_utils
from concourse.masks import make_identity

BF16 = mybir.dt.bfloat16
F32 = mybir.dt.float32
AF = mybir.ActivationFunctionType
ALU = mybir.AluOpType

B, LC, LO, NOPT, E, H = 16, 128, 64, 5, 300, 256
NCORES = 8
BL = B // NCORES          # 2 batch rows per core
NI = BL * NOPT            # 10 (b,opt) pairs per core
NBM = BL + NI             # 12 cols in main GRU (2 ctx + 10 opt)
NBA = 2 * NI              # 20 cols in att GRU (10 actx + 10 aopt)
H3 = 3 * H                # 768
bf = ml_dtypes.bfloat16

_CACHE = {}


def _build():
    nc = bacc.Bacc("TRN2", target_bir_lowering=False, debug=False,
                   num_devices=NCORES)

    d = {}
    d["xtc"] = nc.dram_tensor("xtc", [3, 128, LC * BL], BF16, kind="ExternalInput")
    d["xto"] = nc.dram_tensor("xto", [3, 128, LO * NI], BF16, kind="ExternalInput")
    d["wir"] = nc.dram_tensor("wir", [2, 3, 128, H3], BF16, kind="ExternalInput")
    d["whr"] = nc.dram_tensor("whr", [2, 2, 128, H3], BF16, kind="ExternalInput")
    d["wia"] = nc.dram_tensor("wia", [2, 3, 128, H3], BF16, kind="ExternalInput")
    d["wha"] = nc.dram_tensor("wha", [2, 2, 128, H3], BF16, kind="ExternalInput")
    d["wk"] = nc.dram_tensor("wk", [4, 128, H], BF16, kind="ExternalInput")
    d["wq"] = nc.dram_tensor("wq", [4, 128, H], BF16, kind="ExternalInput")
    d["bhn_r"] = nc.dram_tensor("bhn_r", [128, 2, 2], F32, kind="ExternalInput")
    d["bhn_a"] = nc.dram_tensor("bhn_a", [128, 2, 2], F32, kind="ExternalInput")
    d["v"] = nc.dram_tensor("v", [128, 2], BF16, kind="ExternalInput")
    d["out"] = nc.dram_tensor("out", [1, NI], F32, kind="ExternalOutput")

    with tile.TileContext(nc) as tc:
        _body(nc, tc, d)
    nc.compile()
    return nc


def _body(nc, tc, d):
    import contextlib
    ctx = contextlib.ExitStack()
    with ctx:
        consts = ctx.enter_context(tc.tile_pool(name="consts", bufs=1))
        wpool = ctx.enter_context(tc.tile_pool(name="weights", bufs=1))
        xppool = ctx.enter_context(tc.tile_pool(name="xp", bufs=1))
        encp = ctx.enter_context(tc.tile_pool(name="enc", bufs=1))
        hpool = ctx.enter_context(tc.tile_pool(name="hstate", bufs=1))
        spool = ctx.enter_context(tc.tile_pool(name="spool", bufs=2))
        small = ctx.enter_context(tc.tile_pool(name="small", bufs=3))
        psg = ctx.enter_context(tc.tile_pool(name="psg", bufs=2, space="PSUM"))
        psum_hp = ctx.enter_context(tc.tile_pool(name="pshp", bufs=2, space="PSUM"))
        psum_e = ctx.enter_context(tc.tile_pool(name="pse", bufs=2, space="PSUM"))

        def ps_tile(shape):
            return psg.tile(shape, F32, tag="ps", name="pst")

        # ---- constants / weights ----
        ident = consts.tile([128, 128], BF16)
        make_identity(nc, ident[:])
        ones128 = consts.tile([128, 1], F32)
        nc.vector.memset(ones128[:], 1.0)
        ones_bf = consts.tile([128, 128], BF16)
        nc.vector.memset(ones_bf[:], 1.0)

        wir = wpool.tile([128, 2, 3, H3], BF16)
        whr = wpool.tile([128, 2, 2, H3], BF16)
        wia = wpool.tile([128, 2, 3, H3], BF16)
        wha = wpool.tile([128, 2, 2, H3], BF16)
        wk = wpool.tile([128, 4, H], BF16)
        wq = wpool.tile([128, 4, H], BF16)
        bhn_r = consts.tile([128, 2, 2], F32)
        bhn_a = consts.tile([128, 2, 2], F32)
        vsb = consts.tile([128, 2], BF16)
        for dd in range(2):
            for k in range(3):
                nc.sync.dma_start(wir[:, dd, k, :], d["wir"].ap()[dd, k])
                nc.sync.dma_start(wia[:, dd, k, :], d["wia"].ap()[dd, k])
            for k in range(2):
                nc.sync.dma_start(whr[:, dd, k, :], d["whr"].ap()[dd, k])
                nc.sync.dma_start(wha[:, dd, k, :], d["wha"].ap()[dd, k])
        for k in range(4):
            nc.sync.dma_start(wk[:, k, :], d["wk"].ap()[k])
            nc.sync.dma_start(wq[:, k, :], d["wq"].ap()[k])
        nc.sync.dma_start(bhn_r[:], d["bhn_r"].ap())
        nc.sync.dma_start(bhn_a[:], d["bhn_a"].ap())
        nc.sync.dma_start(vsb[:], d["v"].ap())

        xtc = wpool.tile([128, 3, LC * BL], BF16)
        xto = wpool.tile([128, 3, LO * NI], BF16)
        for k in range(3):
            nc.sync.dma_start(xtc[:, k, :], d["xtc"].ap()[k])
            nc.sync.dma_start(xto[:, k, :], d["xto"].ap()[k])

        # ======== Phase 1: main GRU input projections ========
        # unified xp: [p, dir, gate, t, col]; opt cols zero-padded outside
        # their valid range; bwd opt block stored at t in [64,128) so the
        # uniform bwd index T-1-t_f reads opt time 63-t_f.
        xpm = xppool.tile([128, 2, 6, LC, NBM], BF16, tag="xpu")
        nc.vector.memset(xpm[:, 0, :, LO:, BL:], 0.0)
        nc.vector.memset(xpm[:, 1, :, :LO, BL:], 0.0)

        def proj_main(groups):
            for (xsrc, dd, tb, cl, ch, T2, nbg, tch) in groups:
                for jg in range(6):
                    for t0 in range(0, T2, tch):
                        tw = min(tch, T2 - t0)
                        cw = tw * nbg
                        pt = ps_tile([128, 512])
                        for k in range(3):
                            nc.tensor.matmul(
                                pt[:, :cw],
                                wir[:, dd, k, jg * 128:(jg + 1) * 128],
                                xsrc[:, k, t0 * nbg:t0 * nbg + cw],
                                start=(k == 0), stop=(k == 2))
                        nc.scalar.copy(
                            xpm[:, dd, jg, tb + t0:tb + t0 + tw, cl:ch],
                            pt[:, :cw])

        # NOTE: both directions project from the SAME (unreversed) input; the
        # bwd recurrence consumes xp at index Tb-1-t_f, which walks original
        # time in reverse — the true bwd GRU order.
        proj_main([
            (xtc, 0, 0, 0, BL, LC, BL, 128),
            (xtc, 1, 0, 0, BL, LC, BL, 128),
            (xto, 0, 0, BL, NBM, LO, NI, 32),
            (xto, 1, LO, BL, NBM, LO, NI, 32),
        ])

        # ======== Phase 2/6 shared: one bidirectional GRU time step ========
        def gru_step(t_f, whx, xps, hst, nb, bhn, store):
            """xps: list of (xp_tile, npt, c0, Tb); cols [c0, c0+npt) of the
            batch take input from xp_tile at their own time index."""
            for dd in range(2):
                hp = psum_hp.tile([128, 6, nb], F32, tag="hp")
                for jg in range(6):
                    for k in range(2):
                        nc.tensor.matmul(
                            hp[:, jg, :],
                            whx[:, dd, k, jg * 128:(jg + 1) * 128],
                            hst[:, dd, k, :],
                            start=(k == 0), stop=(k == 1))
                rz = small.tile([128, 4, nb], F32, tag="rz")
                if len(xps) == 1:
                    xp, npt, c0, Tb = xps[0]
                    t2 = t_f if dd == 0 else Tb - 1 - t_f
                    nc.vector.tensor_tensor(
                        rz[:], hp[:, 0:4, :], xp[:, dd, 0:4, t2, :], ALU.add)
                else:
                    nc.vector.tensor_copy(rz[:], hp[:, 0:4, :])
                    for (xp, npt, c0, Tb) in xps:
                        t2 = t_f if dd == 0 else Tb - 1 - t_f
                        if not (0 <= t2 < Tb):
                            continue
                        nc.vector.tensor_tensor(
                            rz[:, :, c0:c0 + npt], rz[:, :, c0:c0 + npt],
                            xp[:, dd, 0:4, t2, :], ALU.add)
                nc.scalar.activation(rz[:], rz[:], AF.Sigmoid)
                nt = small.tile([128, 2, nb], F32, tag="nt")
                nc.vector.tensor_scalar(
                    nt[:, 0, :], hp[:, 4, :], bhn[:, dd, 0:1], None, op0=ALU.add)
                nc.vector.tensor_scalar(
                    nt[:, 1, :], hp[:, 5, :], bhn[:, dd, 1:2], None, op0=ALU.add)
                nc.vector.tensor_tensor(nt[:], rz[:, 0:2, :], nt[:], ALU.mult)
                for (xp, npt, c0, Tb) in xps:
                    t2 = t_f if dd == 0 else Tb - 1 - t_f
                    if len(xps) > 1 and not (0 <= t2 < Tb):
                        continue
                    nc.vector.tensor_tensor(
                        nt[:, :, c0:c0 + npt], nt[:, :, c0:c0 + npt],
                        xp[:, dd, 4:6, t2, :], ALU.add)
                nc.scalar.activation(nt[:], nt[:], AF.Tanh)
                hn = small.tile([128, 2, nb], F32, tag="hn")
                nc.vector.tensor_tensor(hn[:], hst[:, dd, :, :], nt[:],
                                        ALU.subtract)
                nc.vector.tensor_tensor(hn[:], rz[:, 2:4, :], hn[:], ALU.mult)
                nc.vector.tensor_tensor(hst[:, dd, :, :], nt[:], hn[:], ALU.add)
                store(dd, t_f, hst)

        # ======== Phase 2: main GRU recurrence ========
        ence = encp.tile([128, 4, LC, BL], BF16)
        enco = encp.tile([128, 4, LO, NI], BF16)
        hm = hpool.tile([128, 2, 2, NBM], BF16, tag="h")
        nc.vector.memset(hm[:], 0.0)

        def store_main(dd, t_f, hst):
            tc_ = t_f if dd == 0 else LC - 1 - t_f
            nc.vector.tensor_copy(ence[:, 2 * dd:2 * dd + 2, tc_, :],
                                  hst[:, dd, :, 0:BL])
            to = t_f if dd == 0 else LO - 1 - t_f
            if 0 <= to < LO:
                nc.vector.tensor_copy(enco[:, 2 * dd:2 * dd + 2, to, :],
                                      hst[:, dd, :, BL:])

        xps_main = [(xpm, NBM, 0, LC)]
        for t in range(LC):
            gru_step(t, whr, xps_main, hm, NBM, bhn_r, store_main)

        # ======== Phase 3: ctx_key / opt_q projections ========
        ctxkT = encp.tile([128, 2, LC, BL], BF16)
        optqT = encp.tile([128, 2, LO, NI], BF16)

        def kq(dst, w, src, T, nb2, tch):
            for jg in range(2):
                for t0 in range(0, T, tch):
                    cw = (min(tch, T - t0)) * nb2
                    pt = ps_tile([128, 512])
                    for k in range(4):
                        nc.tensor.matmul(
                            pt[:, :cw], w[:, k, jg * 128:(jg + 1) * 128],
                            src[:, k, t0:t0 + min(tch, T - t0), :],
                            start=(k == 0), stop=(k == 3))
                    nc.vector.tensor_copy(
                        dst[:, jg, t0:t0 + min(tch, T - t0), :], pt[:, :cw])

        kq(ctxkT, wk, ence, LC, BL, 128)       # 128*2=256 cols/chunk
        kq(optqT, wq, enco, LO, NI, 32)        # 32*10=320 cols/chunk

        def bf_transpose(dst, src, pcols, ocols):
            """src [pcols, ocols] bf16 -> dst [ocols, pcols] bf16."""
            pt = psg.tile([128, 512], BF16, tag="psbf", name="pst_bf")
            nc.tensor.transpose(pt[:ocols, :pcols], src, ident[:pcols, :pcols])
            nc.vector.tensor_copy(dst, pt[:ocols, :pcols])

        ck_t = encp.tile([128, BL, 2, 128], BF16)
        for b in range(BL):
            for jg in range(2):
                bf_transpose(ck_t[:, b, jg, :], ctxkT[:, jg, :, b], 128, 128)
        ctxk_cb = [[ck_t[:, b, jg, :] for jg in range(2)] for b in range(BL)]
        # opt_q transposed [q, h] for the a_ctx aggregation, hoisted out of
        # the per-pair loop
        oq_t = encp.tile([64, NI, 2, 128], BF16)
        for i in range(NI):
            for jg in range(2):
                bf_transpose(oq_t[:, i, jg, :], optqT[:, jg, :, i], 128, 64)
        oq_all = [[oq_t[:, i, jg, :] for jg in range(2)] for i in range(NI)]

        # ======== Phase 4: attention per (b, opt) ========
        # |e| <= sum|v| ~ 8, so exp() is safe in fp32 without max-subtraction.
        # Both softmaxes share one exp(e): P1 = eu/rowsum (softmax over q,
        # free axis), P2 = eu/colsum (softmax over c, partition axis; the
        # colsum comes from a ones-row matmul and normalization is folded
        # into the aggregation post-scale).
        actxT = encp.tile([128, 2, NI, LC], BF16)
        aoptT = encp.tile([128, 2, NI, LO], BF16)
        QCH = 16
        for b in range(BL):
            for o in range(NOPT):
                i = b * NOPT + o
                e_ps = psum_e.tile([128, LO], F32, tag="e")
                for q0 in range(0, LO, QCH):
                    sts = []
                    for jg in range(2):
                        st = spool.tile([128, QCH, LC], F32, tag=f"s{jg}")
                        nc.vector.tensor_tensor(
                            st[:],
                            optqT[:, jg, q0:q0 + QCH, i:i + 1]
                                .broadcast_to([128, QCH, LC]),
                            ctxkT[:, jg, None, :, b]
                                .broadcast_to([128, QCH, LC]),
                            ALU.add)
                        stb = spool.tile([128, QCH, LC], BF16, tag=f"sb{jg}")
                        nc.scalar.activation(stb[:], st[:], AF.Tanh)
                        sts.append(stb)
                    for q in range(QCH):
                        for jg in range(2):
                            nc.tensor.matmul(
                                e_ps[:, q0 + q:q0 + q + 1],
                                sts[jg][:, q, :], vsb[:, jg:jg + 1],
                                start=(jg == 0), stop=(jg == 1))
                # shared unnormalized exp(e) [c, q]
                eu = small.tile([128, LO], BF16, tag="eu")
                nc.scalar.activation(eu[:], e_ps[:], AF.Exp)
                # P1: softmax over q (free axis)
                sm = small.tile([128, 1], F32, tag="sm")
                nc.vector.tensor_reduce(sm[:], eu[:],
                                        axis=mybir.AxisListType.X, op=ALU.add)
                nc.vector.reciprocal(sm[:], sm[:])
                p1 = small.tile([128, LO], BF16, tag="p1")
                nc.vector.tensor_scalar_mul(p1[:], eu[:], sm[:])
                pt1 = psg.tile([128, 512], BF16, tag="psbf", name="pst_bf")
                nc.tensor.transpose(pt1[:64, :128], p1[:], ident[:])
                p1t = small.tile([64, 128], BF16, tag="p1tb")
                nc.vector.tensor_copy(p1t[:], pt1[:64, :128])
                # P2 colsum over c, replicated across partitions by a
                # ones-matrix matmul; normalize after aggregation
                s2_ps = ps_tile([128, 512])
                nc.tensor.matmul(s2_ps[:, :LO], ones_bf[:], eu[:],
                                 start=True, stop=True)
                r2 = small.tile([128, LO], F32, tag="r2")
                nc.vector.reciprocal(r2[:], s2_ps[:, :LO])
                for jg in range(2):
                    ac_ps = ps_tile([128, 512])
                    nc.tensor.matmul(ac_ps[:, :128], oq_all[i][jg], p1t[:],
                                     start=True, stop=True)
                    nc.vector.tensor_copy(actxT[:, jg, i, :], ac_ps[:, :128])
                    ao_ps = ps_tile([128, 512])
                    nc.tensor.matmul(ao_ps[:, :64], ctxk_cb[b][jg], eu[:],
                                     start=True, stop=True)
                    nc.vector.tensor_tensor(
                        aoptT[:, jg, i, :], ao_ps[:, :64], r2[:], ALU.mult)

        # ======== Phase 5: att GRU input projections ========
        xpac = xppool.tile([128, 2, 6, LC, NI], BF16, tag="xpc")
        xpao = xppool.tile([128, 2, 6, LO, NI], BF16, tag="xpd")
        onesrow = consts.tile([1, LC * NI], BF16)
        nc.vector.memset(onesrow[:], 1.0)

        def proj_att(dst, src, T, tch):
            for dd in range(2):
                for jg in range(6):
                    for t0 in range(0, T, tch):
                        tw = min(tch, T - t0)
                        cw = tw * NI
                        pt = ps_tile([128, 512])
                        for k in range(2):
                            nc.tensor.matmul(
                                pt[:, :cw],
                                wia[:, dd, k, jg * 128:(jg + 1) * 128],
                                src[:, k, t0:t0 + tw, :],
                                start=(k == 0), stop=False)
                        nc.tensor.matmul(
                            pt[:, :cw],
                            wia[0:1, dd, 2, jg * 128:(jg + 1) * 128],
                            onesrow[0:1, :cw],
                            start=False, stop=True)
                        nc.scalar.copy(dst[:, dd, jg, t0:t0 + tw, :],
                                       pt[:, :cw])

        # transposed views [128, k, t, i] of actxT/aoptT ([128, jg, i, t])
        acv = actxT[:].transpose([0, 1, 3, 2])
        aov = aoptT[:].transpose([0, 1, 3, 2])
        proj_att(xpac, acv, LC, 32)
        proj_att(xpao, aov, LO, 32)

        # ======== Phase 6: att GRU recurrence with mean accumulation ========
        ha = hpool.tile([128, 2, 2, NBA], BF16, tag="h")
        nc.vector.memset(ha[:], 0.0)
        acc_c = encp.tile([128, 2, 2, NI], F32)
        acc_o = encp.tile([128, 2, 2, NI], F32)
        nc.vector.memset(acc_c[:], 0.0)
        nc.vector.memset(acc_o[:], 0.0)

        def store_att(dd, t_f, hst):
            nc.vector.tensor_tensor(acc_c[:, dd], acc_c[:, dd],
                                    hst[:, dd, :, 0:NI], ALU.add)
            to = t_f if dd == 0 else LO - 1 - t_f
            if 0 <= to < LO:
                nc.vector.tensor_tensor(acc_o[:, dd], acc_o[:, dd],
                                        hst[:, dd, :, NI:], ALU.add)

        xps_att = [(xpac, NI, 0, LC), (xpao, NI, NI, LO)]
        for t in range(LC):
            gru_step(t, wha, xps_att, ha, NBA, bhn_a, store_att)

        # ======== Phase 7: cosine similarity ========
        nc.vector.tensor_scalar_mul(acc_c[:], acc_c[:], 1.0 / LC)
        nc.vector.tensor_scalar_mul(acc_o[:], acc_o[:], 1.0 / LO)
        prod = small.tile([128, 2, 2, NI], F32, tag="prod")
        dots_ps = psg.tile([1, 3, 4, NI], F32, tag="ps")
        nc.vector.tensor_tensor(prod[:], acc_c[:], acc_o[:], ALU.mult)
        nc.tensor.matmul(dots_ps[:, 0], ones128[:], prod[:],
                         start=True, stop=True)
        nc.vector.tensor_tensor(prod[:], acc_c[:], acc_c[:], ALU.mult)
        nc.tensor.matmul(dots_ps[:, 1], ones128[:], prod[:],
                         start=True, stop=True)
        nc.vector.tensor_tensor(prod[:], acc_o[:], acc_o[:], ALU.mult)
        nc.tensor.matmul(dots_ps[:, 2], ones128[:], prod[:],
                         start=True, stop=True)
        red = small.tile([1, 3, NI], F32, tag="red")
        nc.vector.tensor_reduce(red[:], dots_ps[:].transpose([0, 1, 3, 2]),
                                axis=mybir.AxisListType.X, op=ALU.add)
        nrm = small.tile([1, NI], F32, tag="nrm")
        nc.vector.tensor_tensor(nrm[:], red[:, 1, :], red[:, 2, :], ALU.mult)
        nc.vector.tensor_scalar_max(nrm[:], nrm[:], 1e-30)
        nc.scalar.activation(nrm[:], nrm[:], AF.Sqrt)
        nc.vector.reciprocal(nrm[:], nrm[:])
        cos = small.tile([1, NI], F32, tag="cos")
        nc.vector.tensor_tensor(cos[:], red[:, 0, :], nrm[:], ALU.mult)
        nc.sync.dma_start(d["out"].ap(), cos[:])


def _prep_inputs(inputs):
    ctx = np.asarray(inputs["context"], np.float32)
    opts = np.asarray(inputs["options"], np.float32)

    def gru_w(pre):
        out = {}
        for dd, sfx in enumerate(("f", "b")):
            out[dd] = {k: np.asarray(inputs[f"{pre}_{k}_{sfx}"], np.float32)
                       for k in ("Wi", "Wh", "bi", "bh")}
        return out

    rnn, att = gru_w("rnn"), gru_w("att")
    Wk = np.asarray(inputs["Wk"], np.float32)
    Wq = np.asarray(inputs["Wq"], np.float32)
    v = np.asarray(inputs["v_energy"], np.float32)

    def wi_pack(g, ein):
        out = np.zeros((2, 3, 128, H3), np.float32)
        for dd in range(2):
            bias = g[dd]["bi"].copy()
            bias[:2 * H] += g[dd]["bh"][:2 * H]
            m = np.zeros((3 * 128, H3), np.float32)
            m[:ein] = g[dd]["Wi"].T
            m[ein] = bias
            out[dd] = m.reshape(3, 128, H3)
        return out.astype(bf)

    def wh_pack(g):
        out = np.zeros((2, 2, 128, H3), np.float32)
        for dd in range(2):
            out[dd] = g[dd]["Wh"].T.reshape(2, 128, H3)
        return out.astype(bf)

    def bhn_pack(g):
        out = np.zeros((128, 2, 2), np.float32)
        for dd in range(2):
            out[:, dd, 0] = g[dd]["bh"][2 * H:2 * H + 128]
            out[:, dd, 1] = g[dd]["bh"][2 * H + 128:]
        return out

    shared = {
        "wir": wi_pack(rnn, E), "whr": wh_pack(rnn),
        "wia": wi_pack(att, H), "wha": wh_pack(att),
        "wk": np.ascontiguousarray(Wk.T.reshape(4, 128, H).astype(bf)),
        "wq": np.ascontiguousarray(Wq.T.reshape(4, 128, H).astype(bf)),
        "bhn_r": np.ascontiguousarray(bhn_pack(rnn)),
        "bhn_a": np.ascontiguousarray(bhn_pack(att)),
        "v": np.ascontiguousarray(v.reshape(2, 128).T.astype(bf)),
    }

    in_maps = []
    for c in range(NCORES):
        bs = slice(c * BL, (c + 1) * BL)
        xa = np.zeros((BL, LC, 3 * 128), np.float32)
        xa[:, :, :E] = ctx[bs]
        xa[:, :, E] = 1.0
        xb = np.zeros((NI, LO, 3 * 128), np.float32)
        xb[:, :, :E] = opts[bs].reshape(NI, LO, E)
        xb[:, :, E] = 1.0
        m = dict(shared)
        m["xtc"] = np.ascontiguousarray(
            xa.transpose(2, 1, 0).reshape(3, 128, LC * BL).astype(bf))
        m["xto"] = np.ascontiguousarray(
            xb.transpose(2, 1, 0).reshape(3, 128, LO * NI).astype(bf))
        in_maps.append(m)
    return in_maps


def kernel(**inputs):
    if "nc" not in _CACHE:
        _CACHE["nc"] = _build()
    nc = _CACHE["nc"]
    in_maps = _prep_inputs(inputs)
    res = bass_utils.run_bass_kernel_spmd(nc, in_maps,
                                          core_ids=list(range(NCORES)))
    _CACHE["last_exec_ns"] = res.exec_time_ns
    _CACHE["last_res"] = res
    logits = np.concatenate(
        [np.asarray(res.results[c]["out"], np.float32).reshape(BL, NOPT)
         for c in range(NCORES)], axis=0)
    x = logits - logits.max(axis=1, keepdims=True)
    ex = np.exp(x)
    return (ex / ex.sum(axis=1, keepdims=True)).astype(np.float32)


if __name__ == "__main__":
    _build()
    print("build+compile OK")

